# revision 1
# baseline (speedup 1.0000x reference)
"""AutonomyCost embedding-lookup kernel v3 for 8 TRN2 NeuronCores.

out[b] = sum_l eta[idx[b,l]] + (-0.5*t*log(t+eps)) + trapz(exp(-E), E).

Architecture (per core: 8192 rows x 512 lookups from a 100k fp32 table):
- hi = idx>>2 indexes a [25000, 64]-f32 HBM table (payload = 4 floats per
  256B row); SWDGE dma_gather with forged elem_size=4 (16B/descriptor),
  8192 indices per call, 8 calls per chunk round-robin on 4 SWDGE queues.
- lo = idx&3 selects within the 4-float window on DVE: mask = (iota4 == lo),
  row sum via fused tensor_tensor_reduce.
- tensor_tensor_reduce is rejected by this device, and DVE 2-port ops
  stall SWDGE descriptor generation -> the reduce is split: DVE builds the
  mask and the product (phased between gather groups), and the ACT engine
  (own SBUF ports, never contends with SWDGE) row-sums the product via
  activation-accumulate.
- Scalar prologue (drag integral + scattering) computed on ACT/DVE after
  the main loop, added to every row sum.
"""
from contextlib import ExitStack

import numpy as np

from concourse import bass, mybir
from concourse.bacc import Bacc
from concourse.bass_types import AP
from concourse.bass_utils import run_bass_kernel_spmd

B, L, V = 65536, 512, 100000
NCORES = 8
RB = B // NCORES            # rows per core = 8192
P = 128                     # partitions
RPP = RB // P               # rows per partition = chunks per core = 64
EPS = 1e-9
NQ = 100

E = 4                       # fp32 payload per table row (gather elem)
STRIDE = 64                 # fp32 row stride (256B)
NROW = V // E               # 25000 table rows
NIDX = 8192                 # indices per dma_gather call
NQUEUES = 4
CALLS = 8                   # calls per chunk
MPC = NIDX // P             # 64 slots per partition per call
SPC = CALLS * MPC           # 512 slots per partition per chunk = one row
GW = SPC * E                # gathered fp32 per partition per chunk (2048)
GRP = 8                     # chunks per gather/reduce phase group


def idxcols(nqueues):
    return (CALLS // nqueues) * (NIDX // 16)


def dma_gather_raw(gpsimd, out_ap, in_ap, idxs_ap, num_idxs, elem_size, elem_step,
                   queue_num, single_packet=False):
    self = gpsimd
    _in_ap = self.lower_ap_dma(in_ap, for_custom_bir_dma=True)
    _idxs_ap = self.lower_ap(idxs_ap)
    _out_ap = self.lower_ap(out_ap)
    return self.add_instruction(
        mybir.InstDMAGatherAnt(
            name=self.bass.get_next_instruction_name(),
            ins=[*_in_ap, _idxs_ap, self.lower_val_access(self.to_reg(num_idxs))],
            outs=[_out_ap],
            transpose=False, num_idxs=num_idxs, elem_size=elem_size,
            stride_bytes_256=(elem_step * 4) // 256,
            gen_mode=0, single_packet=single_packet, queue_num=queue_num,
            sbuf_tokens_per_rank=0, sbuf_free_dim_per_rank=0,
            sbuf_free_dim_pad_per_rank=0, sbuf_byte_offset=0,
        )
    )


def build_nc(nchunk=RPP, nqueues=NQUEUES, grp=GRP):
    assert nchunk % grp == 0
    ngrp = nchunk // grp
    IDXCOLS = idxcols(nqueues)
    cpq = CALLS // nqueues            # calls per queue per chunk
    nc = Bacc(num_swdge_queues=nqueues)
    tab_t = nc.declare_dram_parameter("tab", [NROW, STRIDE], mybir.dt.float32, isOutput=False)
    idx_t = nc.declare_dram_parameter("idxw", [P, nchunk * IDXCOLS], mybir.dt.int16, isOutput=False)
    lo_t = nc.declare_dram_parameter("lo", [P, nchunk * SPC], mybir.dt.float32, isOutput=False)
    io_t = nc.declare_dram_parameter("iota4", [P, E], mybir.dt.float32, isOutput=False)
    qp_t = nc.declare_dram_parameter("qp", [P, NQ], mybir.dt.float32, isOutput=False)
    tv_t = nc.declare_dram_parameter("tv", [P, 1], mybir.dt.float32, isOutput=False)
    out_t = nc.declare_dram_parameter("out", [P * nchunk], mybir.dt.float32, isOutput=True)

    stack = ExitStack()
    with stack, nc.Block() as block:
        en = stack.enter_context
        s_v = en(nc.semaphore("s_v"))       # ACT chunk reduces retired
        s_m = en(nc.semaphore("s_m"))       # DVE chunk products retired
        s_ep = en(nc.semaphore("s_ep"))     # epilogue dataflow
        s_out = en(nc.semaphore("s_out"))
        s_idx = [en(nc.semaphore("s_idx0")), en(nc.semaphore("s_idx1"))]
        s_g = []                            # queue-locked gather-done sems
        for q in range(nqueues):
            s_g.append(en(nc.semaphore(f"sgq{q}")))
        # double-buffered by GROUP: group g uses slot g%2
        idx_sb = en(nc.sbuf_tensor("idx_sb", [P, 2, grp * IDXCOLS], mybir.dt.int16))
        lo_sb = en(nc.sbuf_tensor("lo_sb", [P, 2, grp * SPC], mybir.dt.float32))
        g_sb = en(nc.sbuf_tensor("g_sb", [P, grp, GW], mybir.dt.float32))
        m_sb = en(nc.sbuf_tensor("m_sb", [P, 2, GW], mybir.dt.float32))
        pr_sb = en(nc.sbuf_tensor("pr_sb", [P, 4, GW], mybir.dt.float32))
        jk_sb = en(nc.sbuf_tensor("jk_sb", [P, 2, GW], mybir.dt.float32))
        io_sb = en(nc.sbuf_tensor("io_sb", [P, E], mybir.dt.float32))
        red_sb = en(nc.sbuf_tensor("red_sb", [P, nchunk], mybir.dt.float32))
        qp_sb = en(nc.sbuf_tensor("qp_sb", [P, NQ], mybir.dt.float32))
        e_sb = en(nc.sbuf_tensor("e_sb", [P, NQ], mybir.dt.float32))
        mu_sb = en(nc.sbuf_tensor("mu_sb", [P, NQ], mybir.dt.float32))
        t_sb = en(nc.sbuf_tensor("t_sb", [P, 1], mybir.dt.float32))
        ac_sb = en(nc.sbuf_tensor("ac_sb", [P, 1], mybir.dt.float32))
        lg_sb = en(nc.sbuf_tensor("lg_sb", [P, 1], mybir.dt.float32))
        w0_sb = en(nc.sbuf_tensor("w0_sb", [P, 1], mybir.dt.float32))
        w1_sb = en(nc.sbuf_tensor("w1_sb", [P, 1], mybir.dt.float32))
        c_sb = en(nc.sbuf_tensor("c_sb", [P, 1], mybir.dt.float32))

        @block.sync
        def _(sync):
            for g in range(ngrp):
                if g == 1:
                    # epilogue constants: deferred past group 0's loads so
                    # the first gathers start as early as possible
                    sync.dma_start(out=qp_sb[:], in_=qp_t[:]).then_inc(s_ep, 16)
                    sync.dma_start(out=t_sb[:], in_=tv_t[:]).then_inc(s_ep, 16)
                    sync.dma_start(out=io_sb[:], in_=io_t[:]).then_inc(s_ep, 16)
                if g >= 2:
                    # idx/lo slot g%2 free once DVE retired group g-2
                    sync.wait_ge(s_m, (g - 1) * grp)
                sync.dma_start(
                    out=AP(idx_sb, (g % 2) * grp * IDXCOLS,
                           [[2 * grp * IDXCOLS, P], [1, grp * IDXCOLS]]),
                    in_=AP(idx_t, g * grp * IDXCOLS,
                           [[nchunk * IDXCOLS, P], [1, grp * IDXCOLS]]),
                ).then_inc(s_idx[g % 2], 16)
                sync.dma_start(
                    out=AP(lo_sb, (g % 2) * grp * SPC,
                           [[2 * grp * SPC, P], [1, grp * SPC]]),
                    in_=AP(lo_t, g * grp * SPC,
                           [[nchunk * SPC, P], [1, grp * SPC]]),
                ).then_inc(s_idx[g % 2], 16)
            sync.wait_ge(s_v, nchunk + 1)
            sync.dma_start(
                out=AP(out_t, 0, [[nchunk, P], [1, nchunk]]),
                in_=red_sb[:],
            ).then_inc(s_out, 16)
            sync.wait_ge(s_out, 16)

        @block.gpsimd
        def _(gpsimd):
            for g in range(ngrp):
                gpsimd.wait_ge(s_idx[g % 2], 32 * (g // 2 + 1))
                if g >= 1:
                    # phase: DVE done with group g-1 before more SWDGE gen
                    # (2-port DVE ops stall Q7 descriptor writes)
                    gpsimd.wait_ge(s_m, g * grp)
                for kk in range(grp):
                    k = g * grp + kk
                    for j in range(CALLS):
                        r, q = j // nqueues, j % nqueues
                        ioff = ((g % 2) * grp + kk) * IDXCOLS + r * (NIDX // 16)
                        goff = kk * GW + j * MPC * E
                        dma_gather_raw(
                            gpsimd,
                            AP(g_sb, goff, [[grp * GW, P], [E, MPC], [1, E]]),
                            AP(tab_t, 0, [[STRIDE, NROW], [1, E]]),
                            AP(idx_sb, ioff, [[2 * grp * IDXCOLS, P], [1, NIDX // 16]]),
                            NIDX, E, STRIDE, queue_num=q,
                        ).then_inc(s_g[q], 16)

        @block.vector
        def _(vector):
            vector.wait_ge(s_ep, 48)   # io_sb (iota4) + qp/t landed
            for g in range(ngrp):
                for q in range(nqueues):
                    vector.wait_ge(s_g[q], 16 * cpq * grp * (g + 1))
                if g >= 1:
                    # pr slot safety: ACT must have consumed group g-1
                    vector.wait_ge(s_v, g * grp)
                for kk in range(grp):
                    k = g * grp + kk
                    vector.tensor_tensor(
                        out=AP(m_sb, (kk % 2) * GW, [[2 * GW, P], [1, GW]]),
                        in0=AP(lo_sb, ((g % 2) * grp + kk) * SPC,
                               [[2 * grp * SPC, P], [1, SPC], [0, E]]),
                        in1=AP(io_sb, 0, [[E, P], [0, SPC], [1, E]]),
                        op=mybir.AluOpType.is_equal,
                    )
                    vector.drain()
                    vector.tensor_tensor(
                        out=AP(pr_sb, (k % 4) * GW, [[4 * GW, P], [1, GW]]),
                        in0=AP(g_sb, kk * GW, [[grp * GW, P], [1, GW]]),
                        in1=AP(m_sb, (kk % 2) * GW, [[2 * GW, P], [1, GW]]),
                        op=mybir.AluOpType.mult,
                    ).then_inc(s_m, 1)

            # ---- epilogue head on DVE (gathers all done) ----
            vector.wait_ge(s_ep, 48)
            vector.tensor_scalar(
                out=e_sb[:], in0=qp_sb[:], scalar1=t_sb[:, :1], scalar2=None,
                op0=mybir.AluOpType.mult,
            ).then_inc(s_ep, 1)  # -> 49
            vector.tensor_scalar(
                out=w1_sb[:], in0=t_sb[:], scalar1=EPS, scalar2=None,
                op0=mybir.AluOpType.add,
            ).then_inc(s_ep, 1)  # -> 50
            vector.wait_ge(s_ep, 52)   # ACT done: mu, ac, lg
            vector.tensor_tensor(out=w0_sb[:], in0=mu_sb[:, :1],
                                 in1=mu_sb[:, NQ - 1:NQ], op=mybir.AluOpType.add)
            vector.drain()
            vector.tensor_scalar(out=w0_sb[:], in0=w0_sb[:], scalar1=-0.5,
                                 scalar2=None, op0=mybir.AluOpType.mult)
            vector.drain()
            vector.tensor_tensor(out=w0_sb[:], in0=w0_sb[:], in1=ac_sb[:],
                                 op=mybir.AluOpType.add)
            vector.tensor_scalar(out=w1_sb[:], in0=t_sb[:], scalar1=1.0 / (NQ - 1),
                                 scalar2=None, op0=mybir.AluOpType.mult)
            vector.drain()
            vector.tensor_tensor(out=w0_sb[:], in0=w0_sb[:], in1=w1_sb[:],
                                 op=mybir.AluOpType.mult)
            vector.tensor_tensor(out=c_sb[:], in0=lg_sb[:], in1=t_sb[:],
                                 op=mybir.AluOpType.mult)
            vector.drain()
            vector.tensor_scalar(out=c_sb[:], in0=c_sb[:], scalar1=-0.5,
                                 scalar2=None, op0=mybir.AluOpType.mult)
            vector.drain()
            vector.tensor_tensor(out=c_sb[:], in0=c_sb[:], in1=w0_sb[:],
                                 op=mybir.AluOpType.add)
            vector.drain()
            vector.wait_ge(s_v, nchunk)   # ACT reduces all retired
            vector.tensor_scalar(
                out=red_sb[:], in0=red_sb[:], scalar1=c_sb[:, :1], scalar2=None,
                op0=mybir.AluOpType.add,
            ).then_inc(s_v, 1)

        @block.scalar
        def _(scalar):
            for k in range(nchunk):
                scalar.wait_ge(s_m, k + 1)
                scalar.activation(
                    out=AP(jk_sb, (k % 2) * GW, [[2 * GW, P], [1, GW]]),
                    in_=AP(pr_sb, (k % 4) * GW, [[4 * GW, P], [1, GW]]),
                    func=mybir.ActivationFunctionType.Copy,
                    accum_out=red_sb[:, k:k + 1],
                ).then_inc(s_v, 1)
                scalar.drain()
            scalar.wait_ge(s_ep, 50)
            scalar.activation(
                out=mu_sb[:], in_=e_sb[:],
                func=mybir.ActivationFunctionType.Exp,
                scale=-1.0, accum_out=ac_sb[:, :1],
            ).then_inc(s_ep, 1)  # -> 51
            scalar.activation(
                out=lg_sb[:], in_=w1_sb[:],
                func=mybir.ActivationFunctionType.Ln,
            ).then_inc(s_ep, 1)  # -> 52
    nc.compile()
    return nc


def _prep_core(idx_core, nchunk=RPP, nqueues=NQUEUES):
    """idx_core int [P*nchunk, 512] -> (idxw int16 [128, nchunk*IDXCOLS],
    lo fp32 [128, nchunk*512]). Partition p owns rows [nchunk*p, nchunk*(p+1))."""
    IDXCOLS = idxcols(nqueues)
    rounds = CALLS // nqueues
    hi = (np.asarray(idx_core).astype(np.int64) >> 2).astype(np.int16)
    lo = (np.asarray(idx_core).astype(np.int64) & 3).astype(np.float32)
    hi_arr = hi.reshape(P, nchunk * L)
    lo_arr = lo.reshape(P, nchunk * L)
    A = hi_arr.reshape(P, nchunk, CALLS, MPC)         # [p, k, j, m]
    lst = np.transpose(A, (1, 2, 3, 0))               # [k, j, m, p]
    wr = lst.reshape(nchunk, CALLS, NIDX // 16, 16)   # [k, j, f, w]
    wr = np.transpose(wr, (0, 1, 3, 2))               # [k, j, w=16, f=512]
    W2 = wr.reshape(nchunk, rounds, nqueues, 16, NIDX // 16)  # [k, r, q, w, f]
    Bq = np.transpose(W2, (2, 3, 0, 1, 4))            # [q, w, k, r, f]
    Bq = np.concatenate([Bq, Bq], axis=1)             # [q, 32, k, r, f]
    idxw = np.zeros((P, nchunk * IDXCOLS), dtype=np.int16)
    idxw[:32 * nqueues] = Bq.reshape(32 * nqueues, nchunk * IDXCOLS)
    return np.ascontiguousarray(idxw), np.ascontiguousarray(lo_arr)


def make_aux(eta, tval):
    tab = np.zeros((NROW, STRIDE), dtype=np.float32)
    tab[:, :E] = eta.reshape(NROW, E)
    iota = np.tile(np.arange(E, dtype=np.float32)[None, :], (P, 1))
    qp = np.tile(np.linspace(0.0, 1.0, NQ, dtype=np.float32)[None, :], (P, 1))
    tv = np.full((P, 1), tval, dtype=np.float32)
    return {"tab": tab, "iota4": iota, "qp": qp, "tv": tv}


_NC_CACHE = {}


def kernel(decision_indices, eta_table, t):
    idx = np.asarray(decision_indices)
    eta = np.asarray(eta_table, dtype=np.float32)
    tval = float(np.asarray(t, dtype=np.float32))

    aux = make_aux(eta, tval)
    if "nc" not in _NC_CACHE:
        _NC_CACHE["nc"] = build_nc()
    nc = _NC_CACHE["nc"]

    in_maps = []
    for i in range(NCORES):
        idxw, lo = _prep_core(idx[i * RB:(i + 1) * RB])
        in_maps.append({**aux, "idxw": idxw, "lo": lo})
    try:
        res = run_bass_kernel_spmd(nc, in_maps, core_ids=list(range(NCORES)))
        out = np.concatenate(
            [np.asarray(res.results[i]["out"]) for i in range(NCORES)]
        ).astype(np.float32)
        if not np.all(np.isfinite(out)):
            raise RuntimeError("non-finite device output")
        return out
    except Exception as e:
        # Device-path failure (should not happen: validated full-scale on
        # 8 cores). Keep the caller correct, but be loud about it.
        import sys
        import traceback
        print(f"kernel: DEVICE PATH FAILED, numpy fallback: {e!r}", file=sys.stderr)
        traceback.print_exc()
        Eq = (np.arange(NQ, dtype=np.float64) / (NQ - 1)) * tval
        drag = np.trapezoid(np.exp(-Eq), Eq)
        scat = -0.5 * tval * np.log(tval + EPS)
        trace = eta[np.asarray(idx, dtype=np.int64)].sum(axis=1, dtype=np.float64)
        return (trace + scat + drag).astype(np.float32)

